# revision 41
# baseline (speedup 1.0000x reference)
"""Fused multi-head attention block (qkv proj + RoPE + SDPA + out proj) on 8
Trainium2 NeuronCores.

Sharding: data-parallel over batch (4) x tensor-parallel over heads (2 groups
of 8). Core c handles batch c//2, head group c%2. Each core returns a partial
(2048, 1024) output; the host sums the two head-group partials per batch.

All matmul operands are bf16 (fp32 PSUM accumulation). The softmax exp on the
Scalar engine (1 elem/cycle/lane, ~294us for 33.5M elements/core) is the hard
floor; everything else is arranged to hide inside the exp stream:

  - SDPA is blocked by (head pair j, query quarter qv of 512): the scores for
    both heads of the pair land in one [128, 1024] PSUM tile (2 banks) and one
    ACTIVATE exps both. y/denominator accumulators need only 2 banks.
  - PSUM: 4 banks score double-buffer + 2 banks y-accum + 2 banks "aux".
    The v projection, the second half of the q/k projection, and c_proj all
    run in the aux banks, so they fill TensorE gaps without ever stalling the
    score->exp pipeline.
  - c_proj for the first query half runs under the second half's SDPA.

Per-core layouts:
  xts  [C, T] bf16   x[b].T (contraction-major for the PE), 8 x [128, T]
  q/k produced as [f', t] where f' interleaves the RoPE halves; after RoPE the
      DVE writes head-contiguous chunks: chunk j holds heads (2j, 2j+1), head
      rows [e*64, e*64+64) = [o1(32); o2(32)] -> K=64 contiguous for S^T.
  v   [t, h*65+d] with a ones column per head (col h*65+64) so one matmul
      accumulates y^T and the softmax denominator in a single PSUM group.

Self-contained: hardcodes B=4, T=2048, C=1024, H=16, D=64.
"""

import numpy as np

B, T, C = 4, 2048, 1024
H, D = 16, 64
HL = H // 2            # heads per core
N_CORES = 8
ROPE_THETA = 10000.0

_NC = None
DEBUG_TAPS = False


def _build_nc():
    import concourse.mybir as mybir
    import concourse.tile as tile
    from concourse import bacc

    F32 = mybir.dt.float32
    BF16 = mybir.dt.bfloat16
    EXP = mybir.ActivationFunctionType.Exp

    nc = bacc.Bacc("TRN2", target_bir_lowering=False, debug=False, num_devices=N_CORES)

    xt = nc.dram_tensor("xt", [C, T], BF16, kind="ExternalInput")        # x[b].T
    wq = nc.dram_tensor("wq", [C, 512], BF16, kind="ExternalInput")      # [C, f']
    wk = nc.dram_tensor("wk", [C, 512], BF16, kind="ExternalInput")
    wv = nc.dram_tensor("wv", [C, 512], BF16, kind="ExternalInput")      # [C, h*64+d]
    wp = nc.dram_tensor("wp", [512, C], BF16, kind="ExternalInput")      # [h*64+d, o]
    cost = nc.dram_tensor("cost", [128, T], BF16, kind="ExternalInput")
    sint = nc.dram_tensor("sint", [128, T], BF16, kind="ExternalInput")
    out = nc.dram_tensor("out", [T, C], BF16, kind="ExternalOutput")
    if DEBUG_TAPS:
        dbg = {
            "dbg_xts0": nc.dram_tensor("dbg_xts0", [128, T], BF16, kind="ExternalOutput"),
            "dbg_wqs0": nc.dram_tensor("dbg_wqs0", [128, 512], BF16, kind="ExternalOutput"),
            "dbg_qbf0": nc.dram_tensor("dbg_qbf0", [128, T], BF16, kind="ExternalOutput"),
            "dbg_kbf0": nc.dram_tensor("dbg_kbf0", [128, T], BF16, kind="ExternalOutput"),
            "dbg_vbf0": nc.dram_tensor("dbg_vbf0", [128, 520], BF16, kind="ExternalOutput"),
            "dbg_ytf0": nc.dram_tensor("dbg_ytf0", [128, T], BF16, kind="ExternalOutput"),
        }

    with tile.TileContext(nc) as tc:
        with (
            tc.tile_pool(name="persist", bufs=1) as pp,
            tc.tile_pool(name="stg", bufs=3) as stg_pool,
            tc.tile_pool(name="ptp", bufs=6) as pt_pool,
            tc.tile_pool(name="obp", bufs=2) as ob_pool,
            tc.tile_pool(name="nrm", bufs=2) as nrm_pool,
            tc.tile_pool(name="pst", bufs=2, space="PSUM") as st_pool,
            tc.tile_pool(name="pya", bufs=2, space="PSUM") as ya_pool,
            tc.tile_pool(name="paux", bufs=2, space="PSUM") as aux_pool,
        ):
            # ---- persistent SBUF tiles -------------------------------------
            xts = [pp.tile([128, T], BF16, name=f"xts{k}", tag=f"xts{k}") for k in range(8)]
            wqs = [pp.tile([128, 512], BF16, name=f"wqs{k}", tag=f"wqs{k}") for k in range(8)]
            wks = [pp.tile([128, 512], BF16, name=f"wks{k}", tag=f"wks{k}") for k in range(8)]
            wvs = [pp.tile([128, 512], BF16, name=f"wvs{k}", tag=f"wvs{k}") for k in range(8)]
            vbf = [pp.tile([128, 520], BF16, name=f"vbf{t}", tag=f"vbf{t}") for t in range(16)]
            qbf = [pp.tile([128, T], BF16, name=f"qbf{j}", tag=f"qbf{j}") for j in range(4)]
            kbf = [pp.tile([128, T], BF16, name=f"kbf{j}", tag=f"kbf{j}") for j in range(4)]
            wps = [pp.tile([128, C], BF16, name=f"wps{c}", tag=f"wps{c}") for c in range(4)]
            ytf = [pp.tile([128, T], BF16, name=f"ytf{c}", tag=f"ytf{c}") for c in range(4)]
            ct = pp.tile([128, T], BF16, name="ct", tag="ct")
            st_ = pp.tile([128, T], BF16, name="st_", tag="st_")

            # ---- input DMAs (wq first: q-proj is the first PE work) --------
            for k in range(8):
                nc.sync.dma_start(wqs[k][:], wq[k * 128:(k + 1) * 128, :])
            for k in range(8):
                nc.sync.dma_start(xts[k][:], xt[k * 128:(k + 1) * 128, :])
            nc.sync.dma_start(ct[:], cost[:])
            nc.sync.dma_start(st_[:], sint[:])
            for k in range(8):
                nc.sync.dma_start(wvs[k][:], wv[k * 128:(k + 1) * 128, :])
            for k in range(8):
                nc.sync.dma_start(wks[k][:], wk[k * 128:(k + 1) * 128, :])
            for c in range(4):
                nc.sync.dma_start(wps[c][:], wp[c * 128:(c + 1) * 128, :])

            # ---- projection helpers ----------------------------------------
            def project_chunk(ws, c, nm, copy_eng, pool, piece):
                """q/k projection for one 128-col chunk -> staged bf16 [128, T].

                `piece` is the PSUM tile width (1024 via the score slots before
                SDPA starts, 512 via the aux slots under the exp stream)."""
                stg = stg_pool.tile([128, T], BF16, name=f"stg_{nm}{c}", tag="stg")
                for p0 in range(T // piece):
                    ps = pool.tile([128, piece], F32, name=f"ps_{nm}{c}_{p0}",
                                   tag="st" if piece == 1024 else "aux")
                    for k in range(8):
                        for n in range(piece // 512):
                            nc.tensor.matmul(
                                ps[:, n * 512:(n + 1) * 512],
                                ws[k][:, c * 128:(c + 1) * 128],
                                xts[k][:, p0 * piece + n * 512: p0 * piece + (n + 1) * 512],
                                start=(k == 0), stop=(k == 7),
                            )
                    copy_eng(stg[:, p0 * piece:(p0 + 1) * piece], ps[:])
                return stg

            def rope_pair(x1, x2, lo, dst, nm):
                # o1 = x1*cos - x2*sin ; o2 = x1*sin + x2*cos   (all bf16)
                # head h window w=(h%4)*32 -> dst[h//2][(h%2)*64 (+32 for o2)]
                a = stg_pool.tile([128, T], BF16, name=f"ra_{nm}", tag="tmp")
                nc.vector.tensor_mul(a[:], x1[:], ct[:])
                b = stg_pool.tile([128, T], BF16, name=f"rb_{nm}", tag="tmp")
                nc.vector.tensor_mul(b[:], x2[:], st_[:])
                for hh in range(4):
                    h = lo * 4 + hh
                    w, j, e = hh * 32, h // 2, h % 2
                    nc.vector.tensor_sub(
                        dst[j][e * 64:e * 64 + 32, :], a[w:w + 32, :], b[w:w + 32, :]
                    )
                c2 = stg_pool.tile([128, T], BF16, name=f"rc_{nm}", tag="tmp")
                nc.vector.tensor_mul(c2[:], x1[:], st_[:])
                d = stg_pool.tile([128, T], BF16, name=f"rd_{nm}", tag="tmp")
                nc.vector.tensor_mul(d[:], x2[:], ct[:])
                for hh in range(4):
                    h = lo * 4 + hh
                    w, j, e = hh * 32, h // 2, h % 2
                    nc.vector.tensor_add(
                        dst[j][e * 64 + 32:e * 64 + 64, :], c2[w:w + 32, :], d[w:w + 32, :]
                    )

            def project_lo(lo, copy_eng, pool, piece):
                x1 = project_chunk(wqs, lo, "q", copy_eng, pool, piece)
                x2 = project_chunk(wqs, 2 + lo, "q", copy_eng, pool, piece)
                rope_pair(x1, x2, lo, qbf, f"q{lo}")
                x1 = project_chunk(wks, lo, "k", copy_eng, pool, piece)
                x2 = project_chunk(wks, 2 + lo, "k", copy_eng, pool, piece)
                rope_pair(x1, x2, lo, kbf, f"k{lo}")

            def v_phase():
                for tm in range(16):
                    vps = aux_pool.tile([128, 512], F32, name=f"vps{tm}", tag="aux")
                    for k in range(8):
                        nc.tensor.matmul(
                            vps[:],
                            xts[k][:, tm * 128:(tm + 1) * 128],
                            wvs[k][:],
                            start=(k == 0), stop=(k == 7),
                        )
                    va = vbf[tm][:].rearrange("p (h x) -> p h x", x=65)
                    nc.scalar.copy(va[:, :, 0:64], vps[:].rearrange("p (h d) -> p h d", d=64))
                    nc.vector.memset(va[:, :, 64], 1.0)

            # ---- SDPA block for head pair j, query quarter qv ---------------
            def sdpa_block(j, qv, tail=False):
                q0 = qv * 512
                yas = [
                    ya_pool.tile([65, 512], F32, name=f"ya_j{j}v{qv}e{e}", tag="ya")
                    for e in range(2)
                ]
                for kc in range(16):
                    stt = st_pool.tile(
                        [128, 1024], F32, name=f"st_j{j}v{qv}k{kc}", tag="st"
                    )
                    for e in range(2):
                        nc.tensor.matmul(
                            stt[:, e * 512:(e + 1) * 512],
                            kbf[j][e * 64:e * 64 + 64, kc * 128:(kc + 1) * 128],
                            qbf[j][e * 64:e * 64 + 64, q0:q0 + 512],
                            start=True, stop=True,
                            tile_position=(e * 64, 0),
                        )
                    pt_t = pt_pool.tile(
                        [128, 1024], BF16, name=f"pt_j{j}v{qv}k{kc}", tag="pt"
                    )
                    nc.scalar.activation(pt_t[:], stt[:], EXP, scale=0.125)
                    for e in range(2):
                        h = 2 * j + e
                        nc.tensor.matmul(
                            yas[e][:],
                            vbf[kc][:, h * 65:(h + 1) * 65],
                            pt_t[:, e * 512:(e + 1) * 512],
                            start=(kc == 0), stop=(kc == 15),
                        )
                for e in range(2):
                    nm2 = f"j{j}v{qv}e{e}"
                    copy_eng = nc.scalar.copy if tail else nc.vector.tensor_copy
                    ya_sb = nrm_pool.tile([64, 512], F32, name=f"yb_{nm2}", tag="yasb")
                    copy_eng(ya_sb[:], yas[e][0:64, :])
                    den = nrm_pool.tile([1, 512], F32, name=f"den_{nm2}", tag="den")
                    copy_eng(den[:], yas[e][64:65, :])
                    rden = nrm_pool.tile([1, 512], F32, name=f"rden_{nm2}", tag="rden")
                    nc.vector.reciprocal_approx_fast(rden[:], den[:])
                    bden = nrm_pool.tile([64, 512], F32, name=f"bden_{nm2}", tag="bden")
                    nc.gpsimd.partition_broadcast(bden[:], rden[:])
                    nc.vector.tensor_mul(
                        ytf[j][e * 64:e * 64 + 64, q0:q0 + 512], ya_sb[:], bden[:]
                    )

            # ---- c_proj for one query quarter (4 qm chunks) -----------------
            def cproj_quarter(qv, pools, tail=False):
                for qm in range(qv * 4, qv * 4 + 4):
                    for oh in range(2):
                        pool, tag = pools[(qm * 2 + oh) % len(pools)]
                        cp = pool.tile([128, 512], F32, name=f"cp{qm}_{oh}", tag=tag)
                        for c in range(4):
                            nc.tensor.matmul(
                                cp[:],
                                ytf[c][:, qm * 128:(qm + 1) * 128],
                                wps[c][:, oh * 512:(oh + 1) * 512],
                                start=(c == 0), stop=(c == 3),
                            )
                        ob = ob_pool.tile([128, 512], BF16, name=f"ob{qm}_{oh}", tag="ob")
                        if tail and (qm + oh) % 2 == 0:
                            nc.scalar.copy(ob[:], cp[:])
                        else:
                            nc.vector.tensor_copy(ob[:], cp[:])
                        nc.sync.dma_start(
                            out[qm * 128:(qm + 1) * 128, oh * 512:(oh + 1) * 512], ob[:]
                        )

            # ---- program order (dataflow) + scheduler priorities ------------
            # Emission must follow dataflow (Tile deps are trace-order), so
            # v/lo1/c_proj(half 0) are emitted at their producer positions but
            # DEMOTED to gap-filler priority: TensorE prefers the score/attnV
            # stream that feeds the Scalar engine's exp pipeline, and fills
            # its idle cycles with the demoted work (which lives in the aux
            # PSUM banks, so it never steals the score buffers).
            LOW = -1_000_000
            project_lo(0, nc.scalar.copy, st_pool, 1024)
            with tc.high_priority(LOW):
                v_phase()
                project_lo(1, nc.vector.tensor_copy, aux_pool, 512)
            sdpa_block(0, 0)
            sdpa_block(1, 0)
            sdpa_block(0, 1)
            sdpa_block(1, 1)
            sdpa_block(0, 2)
            sdpa_block(1, 2)
            sdpa_block(0, 3)
            sdpa_block(1, 3)
            sdpa_block(2, 0)
            sdpa_block(3, 0)
            with tc.high_priority(LOW):
                cproj_quarter(0, [(aux_pool, "aux")])
            sdpa_block(2, 1)
            sdpa_block(3, 1)
            with tc.high_priority(LOW):
                cproj_quarter(1, [(aux_pool, "aux")])
            sdpa_block(2, 2)
            sdpa_block(3, 2)
            with tc.high_priority(LOW):
                cproj_quarter(2, [(aux_pool, "aux")])
            sdpa_block(2, 3)
            sdpa_block(3, 3, tail=True)
            cproj_quarter(3, [(st_pool, "st"), (aux_pool, "aux")], tail=True)

            if DEBUG_TAPS:
                nc.sync.dma_start(dbg["dbg_xts0"][:], xts[0][:])
                nc.sync.dma_start(dbg["dbg_wqs0"][:], wqs[0][:])
                nc.sync.dma_start(dbg["dbg_qbf0"][:], qbf[0][:])
                nc.sync.dma_start(dbg["dbg_kbf0"][:], kbf[0][:])
                nc.sync.dma_start(dbg["dbg_vbf0"][:], vbf[0][:])
                nc.sync.dma_start(dbg["dbg_ytf0"][:], ytf[0][:])

    nc.compile()
    return nc


def _qk_perm():
    """f' (0..511) -> within-group feature index (h*64 + d) for q/k.

    f' = half*256 + (h//4)*128 + (h%4)*32 + i maps to d = 2*i + half.
    """
    perm = np.zeros(512, dtype=np.int64)
    for h in range(HL):
        for i in range(32):
            perm[(h // 4) * 128 + (h % 4) * 32 + i] = h * 64 + 2 * i
            perm[256 + (h // 4) * 128 + (h % 4) * 32 + i] = h * 64 + 2 * i + 1
    return perm


def _rope_tables():
    import ml_dtypes

    i = np.arange(128) % 32
    inv = (1.0 / (ROPE_THETA ** (np.arange(0, D, 2, dtype=np.float32) / D))).astype(np.float32)
    ang = np.arange(T, dtype=np.float32)[None, :] * inv[i][:, None]
    return (
        np.cos(ang).astype(ml_dtypes.bfloat16),
        np.sin(ang).astype(ml_dtypes.bfloat16),
    )


def make_in_maps(x, w_attn, w_proj):
    import ml_dtypes

    bf = ml_dtypes.bfloat16
    x = np.asarray(x, dtype=np.float32)
    w_attn = np.asarray(w_attn, dtype=np.float32)
    w_proj = np.asarray(w_proj, dtype=np.float32)
    perm = _qk_perm()
    cost, sint = _rope_tables()
    in_maps = []
    xts = [np.ascontiguousarray(x[b].T.astype(bf)) for b in range(B)]
    for core in range(N_CORES):
        b, g = core // 2, core % 2
        base = g * 512
        wqc = np.ascontiguousarray(w_attn[base + perm, :].T.astype(bf))
        wkc = np.ascontiguousarray(w_attn[C + base + perm, :].T.astype(bf))
        wvc = np.ascontiguousarray(w_attn[2 * C + base:2 * C + base + 512, :].T.astype(bf))
        wpc = np.ascontiguousarray(w_proj[:, base:base + 512].T.astype(bf))
        in_maps.append(
            {"xt": xts[b], "wq": wqc, "wk": wkc, "wv": wvc, "wp": wpc, "cost": cost, "sint": sint}
        )
    return in_maps


def kernel(x, w_attn, w_proj):
    global _NC
    from concourse.bass_utils import run_bass_kernel_spmd

    if _NC is None:
        _NC = _build_nc()
    in_maps = make_in_maps(x, w_attn, w_proj)
    res = run_bass_kernel_spmd(_NC, in_maps, list(range(N_CORES))).results
    out = np.empty((B, T, C), dtype=np.float32)
    for b in range(B):
        out[b] = res[2 * b]["out"].astype(np.float32) + res[2 * b + 1]["out"].astype(
            np.float32
        )
    return out


# revision 42
# speedup vs baseline: 1.1731x; 1.1731x over previous
"""Fused multi-head attention block (qkv proj + RoPE + SDPA + out proj) on 8
Trainium2 NeuronCores.

Sharding: data-parallel over batch (4) x tensor-parallel over heads (2 groups
of 8). Core c handles batch c//2, head group c%2. Each core returns a partial
(2048, 1024) output; the host sums the two head-group partials per batch.

All matmul operands are bf16 (fp32 PSUM accumulation). The softmax exp on the
Scalar engine (1 elem/cycle/lane, ~294us for 33.5M elements/core) is the hard
floor; everything else is arranged to hide inside the exp stream:

  - SDPA is blocked by (head pair j, query quarter qv of 512): the scores for
    both heads of the pair land in one [128, 1024] PSUM tile (2 banks) and one
    ACTIVATE exps both. y/denominator accumulators need only 2 banks.
  - PSUM: 4 banks score double-buffer + 2 banks y-accum + 2 banks "aux".
    The v projection, the second half of the q/k projection, and c_proj all
    run in the aux banks, so they fill TensorE gaps without ever stalling the
    score->exp pipeline.
  - c_proj for the first query half runs under the second half's SDPA.

Per-core layouts:
  xts  [C, T] bf16   x[b].T (contraction-major for the PE), 8 x [128, T]
  q/k produced as [f', t] where f' interleaves the RoPE halves; after RoPE the
      DVE writes head-contiguous chunks: chunk j holds heads (2j, 2j+1), head
      rows [e*64, e*64+64) = [o1(32); o2(32)] -> K=64 contiguous for S^T.
  v   [t, h*65+d] with a ones column per head (col h*65+64) so one matmul
      accumulates y^T and the softmax denominator in a single PSUM group.

Self-contained: hardcodes B=4, T=2048, C=1024, H=16, D=64.
"""

import numpy as np

B, T, C = 4, 2048, 1024
H, D = 16, 64
HL = H // 2            # heads per core
N_CORES = 8
ROPE_THETA = 10000.0

_NC = None
DEBUG_TAPS = False


def _build_nc():
    import concourse.mybir as mybir
    import concourse.tile as tile
    from concourse import bacc

    F32 = mybir.dt.float32
    BF16 = mybir.dt.bfloat16
    EXP = mybir.ActivationFunctionType.Exp

    nc = bacc.Bacc("TRN2", target_bir_lowering=False, debug=False, num_devices=N_CORES)

    xt = nc.dram_tensor("xt", [C, T], BF16, kind="ExternalInput")        # x[b].T
    wq = nc.dram_tensor("wq", [C, 512], BF16, kind="ExternalInput")      # [C, f']
    wk = nc.dram_tensor("wk", [C, 512], BF16, kind="ExternalInput")
    wv = nc.dram_tensor("wv", [C, 512], BF16, kind="ExternalInput")      # [C, h*64+d]
    wp = nc.dram_tensor("wp", [512, C], BF16, kind="ExternalInput")      # [h*64+d, o]
    cost = nc.dram_tensor("cost", [128, T], BF16, kind="ExternalInput")
    sint = nc.dram_tensor("sint", [128, T], BF16, kind="ExternalInput")
    out = nc.dram_tensor("out", [T, C], BF16, kind="ExternalOutput")
    if DEBUG_TAPS:
        dbg = {
            "dbg_xts0": nc.dram_tensor("dbg_xts0", [128, T], BF16, kind="ExternalOutput"),
            "dbg_wqs0": nc.dram_tensor("dbg_wqs0", [128, 512], BF16, kind="ExternalOutput"),
            "dbg_qbf0": nc.dram_tensor("dbg_qbf0", [128, T], BF16, kind="ExternalOutput"),
            "dbg_kbf0": nc.dram_tensor("dbg_kbf0", [128, T], BF16, kind="ExternalOutput"),
            "dbg_vbf0": nc.dram_tensor("dbg_vbf0", [128, 520], BF16, kind="ExternalOutput"),
            "dbg_ytf0": nc.dram_tensor("dbg_ytf0", [128, T], BF16, kind="ExternalOutput"),
        }

    with tile.TileContext(nc) as tc:
        with (
            tc.tile_pool(name="persist", bufs=1) as pp,
            tc.tile_pool(name="stg", bufs=3) as stg_pool,
            tc.tile_pool(name="ptp", bufs=6) as pt_pool,
            tc.tile_pool(name="obp", bufs=2) as ob_pool,
            tc.tile_pool(name="nrm", bufs=2) as nrm_pool,
            tc.tile_pool(name="pst", bufs=2, space="PSUM") as st_pool,
            tc.tile_pool(name="pya", bufs=2, space="PSUM") as ya_pool,
            tc.tile_pool(name="paux", bufs=2, space="PSUM") as aux_pool,
        ):
            # ---- persistent SBUF tiles -------------------------------------
            xts = [pp.tile([128, T], BF16, name=f"xts{k}", tag=f"xts{k}") for k in range(8)]
            wqs = [pp.tile([128, 512], BF16, name=f"wqs{k}", tag=f"wqs{k}") for k in range(8)]
            wks = [pp.tile([128, 512], BF16, name=f"wks{k}", tag=f"wks{k}") for k in range(8)]
            wvs = [pp.tile([128, 512], BF16, name=f"wvs{k}", tag=f"wvs{k}") for k in range(8)]
            vbf = [pp.tile([128, 520], BF16, name=f"vbf{t}", tag=f"vbf{t}") for t in range(16)]
            qbf = [pp.tile([128, T], BF16, name=f"qbf{j}", tag=f"qbf{j}") for j in range(4)]
            kbf = [pp.tile([128, T], BF16, name=f"kbf{j}", tag=f"kbf{j}") for j in range(4)]
            wps = [pp.tile([128, C], BF16, name=f"wps{c}", tag=f"wps{c}") for c in range(4)]
            ytf = [pp.tile([128, T], BF16, name=f"ytf{c}", tag=f"ytf{c}") for c in range(4)]
            ct = pp.tile([128, T], BF16, name="ct", tag="ct")
            st_ = pp.tile([128, T], BF16, name="st_", tag="st_")

            # ---- input DMAs (wq first: q-proj is the first PE work) --------
            for k in range(8):
                nc.sync.dma_start(wqs[k][:], wq[k * 128:(k + 1) * 128, :])
            for k in range(8):
                nc.sync.dma_start(xts[k][:], xt[k * 128:(k + 1) * 128, :])
            for k in range(8):
                nc.sync.dma_start(wvs[k][:], wv[k * 128:(k + 1) * 128, :])
            for k in range(8):
                nc.sync.dma_start(wks[k][:], wk[k * 128:(k + 1) * 128, :])
            nc.sync.dma_start(ct[:], cost[:])
            nc.sync.dma_start(st_[:], sint[:])
            for c in range(4):
                nc.sync.dma_start(wps[c][:], wp[c * 128:(c + 1) * 128, :])

            # ---- projection helpers ----------------------------------------
            def project_chunk(ws, c, nm, copy_eng, pool, piece):
                """q/k projection for one 128-col chunk -> staged bf16 [128, T].

                `piece` is the PSUM tile width (1024 via the score slots before
                SDPA starts, 512 via the aux slots under the exp stream)."""
                stg = stg_pool.tile([128, T], BF16, name=f"stg_{nm}{c}", tag="stg")
                for p0 in range(T // piece):
                    ps = pool.tile([128, piece], F32, name=f"ps_{nm}{c}_{p0}",
                                   tag="st" if piece == 1024 else "aux")
                    for k in range(8):
                        for n in range(piece // 512):
                            nc.tensor.matmul(
                                ps[:, n * 512:(n + 1) * 512],
                                ws[k][:, c * 128:(c + 1) * 128],
                                xts[k][:, p0 * piece + n * 512: p0 * piece + (n + 1) * 512],
                                start=(k == 0), stop=(k == 7),
                            )
                    copy_eng(stg[:, p0 * piece:(p0 + 1) * piece], ps[:])
                return stg

            def rope_pair(x1, x2, lo, dst, nm):
                # o1 = x1*cos - x2*sin ; o2 = x1*sin + x2*cos   (all bf16)
                # head h window w=(h%4)*32 -> dst[h//2][(h%2)*64 (+32 for o2)]
                a = stg_pool.tile([128, T], BF16, name=f"ra_{nm}", tag="tmp")
                nc.vector.tensor_mul(a[:], x1[:], ct[:])
                b = stg_pool.tile([128, T], BF16, name=f"rb_{nm}", tag="tmp")
                nc.vector.tensor_mul(b[:], x2[:], st_[:])
                for hh in range(4):
                    h = lo * 4 + hh
                    w, j, e = hh * 32, h // 2, h % 2
                    nc.vector.tensor_sub(
                        dst[j][e * 64:e * 64 + 32, :], a[w:w + 32, :], b[w:w + 32, :]
                    )
                c2 = stg_pool.tile([128, T], BF16, name=f"rc_{nm}", tag="tmp")
                nc.vector.tensor_mul(c2[:], x1[:], st_[:])
                d = stg_pool.tile([128, T], BF16, name=f"rd_{nm}", tag="tmp")
                nc.vector.tensor_mul(d[:], x2[:], ct[:])
                for hh in range(4):
                    h = lo * 4 + hh
                    w, j, e = hh * 32, h // 2, h % 2
                    nc.vector.tensor_add(
                        dst[j][e * 64 + 32:e * 64 + 64, :], c2[w:w + 32, :], d[w:w + 32, :]
                    )

            def project_lo(lo, copy_eng, pool, piece):
                x1 = project_chunk(wqs, lo, "q", copy_eng, pool, piece)
                x2 = project_chunk(wqs, 2 + lo, "q", copy_eng, pool, piece)
                rope_pair(x1, x2, lo, qbf, f"q{lo}")
                x1 = project_chunk(wks, lo, "k", copy_eng, pool, piece)
                x2 = project_chunk(wks, 2 + lo, "k", copy_eng, pool, piece)
                rope_pair(x1, x2, lo, kbf, f"k{lo}")

            def v_phase():
                for tm in range(16):
                    vps = aux_pool.tile([128, 512], F32, name=f"vps{tm}", tag="aux")
                    for k in range(8):
                        nc.tensor.matmul(
                            vps[:],
                            xts[k][:, tm * 128:(tm + 1) * 128],
                            wvs[k][:],
                            start=(k == 0), stop=(k == 7),
                        )
                    va = vbf[tm][:].rearrange("p (h x) -> p h x", x=65)
                    nc.scalar.copy(va[:, :, 0:64], vps[:].rearrange("p (h d) -> p h d", d=64))
                    nc.vector.memset(va[:, :, 64], 1.0)

            # ---- SDPA block for head pair j, query quarter qv ---------------
            def sdpa_block(j, qv, tail=False):
                q0 = qv * 512
                yas = [
                    ya_pool.tile([65, 512], F32, name=f"ya_j{j}v{qv}e{e}", tag="ya")
                    for e in range(2)
                ]
                for kc in range(16):
                    stt = st_pool.tile(
                        [128, 1024], F32, name=f"st_j{j}v{qv}k{kc}", tag="st"
                    )
                    for e in range(2):
                        nc.tensor.matmul(
                            stt[:, e * 512:(e + 1) * 512],
                            kbf[j][e * 64:e * 64 + 64, kc * 128:(kc + 1) * 128],
                            qbf[j][e * 64:e * 64 + 64, q0:q0 + 512],
                            start=True, stop=True,
                            tile_position=(e * 64, 0),
                        )
                    pt_t = pt_pool.tile(
                        [128, 1024], BF16, name=f"pt_j{j}v{qv}k{kc}", tag="pt"
                    )
                    nc.scalar.activation(pt_t[:], stt[:], EXP, scale=0.125)
                    for e in range(2):
                        h = 2 * j + e
                        nc.tensor.matmul(
                            yas[e][:],
                            vbf[kc][:, h * 65:(h + 1) * 65],
                            pt_t[:, e * 512:(e + 1) * 512],
                            start=(kc == 0), stop=(kc == 15),
                        )
                for e in range(2):
                    nm2 = f"j{j}v{qv}e{e}"
                    copy_eng = nc.scalar.copy if tail else nc.vector.tensor_copy
                    ya_sb = nrm_pool.tile([64, 512], F32, name=f"yb_{nm2}", tag="yasb")
                    copy_eng(ya_sb[:], yas[e][0:64, :])
                    den = nrm_pool.tile([1, 512], F32, name=f"den_{nm2}", tag="den")
                    copy_eng(den[:], yas[e][64:65, :])
                    rden = nrm_pool.tile([1, 512], F32, name=f"rden_{nm2}", tag="rden")
                    nc.vector.reciprocal_approx_fast(rden[:], den[:])
                    bden = nrm_pool.tile([64, 512], F32, name=f"bden_{nm2}", tag="bden")
                    nc.gpsimd.partition_broadcast(bden[:], rden[:])
                    nc.vector.tensor_mul(
                        ytf[j][e * 64:e * 64 + 64, q0:q0 + 512], ya_sb[:], bden[:]
                    )

            # ---- c_proj for one query quarter (4 qm chunks) -----------------
            def cproj_quarter(qv, pools, tail=False):
                for qm in range(qv * 4, qv * 4 + 4):
                    for oh in range(2):
                        pool, tag = pools[(qm * 2 + oh) % len(pools)]
                        cp = pool.tile([128, 512], F32, name=f"cp{qm}_{oh}", tag=tag)
                        for c in range(4):
                            nc.tensor.matmul(
                                cp[:],
                                ytf[c][:, qm * 128:(qm + 1) * 128],
                                wps[c][:, oh * 512:(oh + 1) * 512],
                                start=(c == 0), stop=(c == 3),
                            )
                        ob = ob_pool.tile([128, 512], BF16, name=f"ob{qm}_{oh}", tag="ob")
                        if tail and (qm + oh) % 2 == 0:
                            nc.scalar.copy(ob[:], cp[:])
                        else:
                            nc.vector.tensor_copy(ob[:], cp[:])
                        nc.sync.dma_start(
                            out[qm * 128:(qm + 1) * 128, oh * 512:(oh + 1) * 512], ob[:]
                        )

            # ---- program order (dataflow) + scheduler priorities ------------
            # Emission must follow dataflow (Tile deps are trace-order), so
            # v/lo1/c_proj(half 0) are emitted at their producer positions but
            # DEMOTED to gap-filler priority: TensorE prefers the score/attnV
            # stream that feeds the Scalar engine's exp pipeline, and fills
            # its idle cycles with the demoted work (which lives in the aux
            # PSUM banks, so it never steals the score buffers).
            LOW = -1_000_000
            project_lo(0, nc.scalar.copy, st_pool, 1024)
            with tc.high_priority(LOW):
                v_phase()
                project_lo(1, nc.vector.tensor_copy, aux_pool, 512)
            sdpa_block(0, 0)
            sdpa_block(1, 0)
            sdpa_block(0, 1)
            sdpa_block(1, 1)
            sdpa_block(0, 2)
            sdpa_block(1, 2)
            sdpa_block(0, 3)
            sdpa_block(1, 3)
            sdpa_block(2, 0)
            sdpa_block(3, 0)
            with tc.high_priority(LOW):
                cproj_quarter(0, [(aux_pool, "aux")])
            sdpa_block(2, 1)
            sdpa_block(3, 1)
            with tc.high_priority(LOW):
                cproj_quarter(1, [(aux_pool, "aux")])
            sdpa_block(2, 2)
            sdpa_block(3, 2)
            with tc.high_priority(LOW):
                cproj_quarter(2, [(aux_pool, "aux")])
            sdpa_block(2, 3)
            sdpa_block(3, 3, tail=True)
            cproj_quarter(3, [(st_pool, "st"), (aux_pool, "aux")], tail=True)

            if DEBUG_TAPS:
                nc.sync.dma_start(dbg["dbg_xts0"][:], xts[0][:])
                nc.sync.dma_start(dbg["dbg_wqs0"][:], wqs[0][:])
                nc.sync.dma_start(dbg["dbg_qbf0"][:], qbf[0][:])
                nc.sync.dma_start(dbg["dbg_kbf0"][:], kbf[0][:])
                nc.sync.dma_start(dbg["dbg_vbf0"][:], vbf[0][:])
                nc.sync.dma_start(dbg["dbg_ytf0"][:], ytf[0][:])

    nc.compile()
    return nc


def _qk_perm():
    """f' (0..511) -> within-group feature index (h*64 + d) for q/k.

    f' = half*256 + (h//4)*128 + (h%4)*32 + i maps to d = 2*i + half.
    """
    perm = np.zeros(512, dtype=np.int64)
    for h in range(HL):
        for i in range(32):
            perm[(h // 4) * 128 + (h % 4) * 32 + i] = h * 64 + 2 * i
            perm[256 + (h // 4) * 128 + (h % 4) * 32 + i] = h * 64 + 2 * i + 1
    return perm


def _rope_tables():
    import ml_dtypes

    i = np.arange(128) % 32
    inv = (1.0 / (ROPE_THETA ** (np.arange(0, D, 2, dtype=np.float32) / D))).astype(np.float32)
    ang = np.arange(T, dtype=np.float32)[None, :] * inv[i][:, None]
    return (
        np.cos(ang).astype(ml_dtypes.bfloat16),
        np.sin(ang).astype(ml_dtypes.bfloat16),
    )


def make_in_maps(x, w_attn, w_proj):
    import ml_dtypes

    bf = ml_dtypes.bfloat16
    x = np.asarray(x, dtype=np.float32)
    w_attn = np.asarray(w_attn, dtype=np.float32)
    w_proj = np.asarray(w_proj, dtype=np.float32)
    perm = _qk_perm()
    cost, sint = _rope_tables()
    in_maps = []
    xts = [np.ascontiguousarray(x[b].T.astype(bf)) for b in range(B)]
    for core in range(N_CORES):
        b, g = core // 2, core % 2
        base = g * 512
        wqc = np.ascontiguousarray(w_attn[base + perm, :].T.astype(bf))
        wkc = np.ascontiguousarray(w_attn[C + base + perm, :].T.astype(bf))
        wvc = np.ascontiguousarray(w_attn[2 * C + base:2 * C + base + 512, :].T.astype(bf))
        wpc = np.ascontiguousarray(w_proj[:, base:base + 512].T.astype(bf))
        in_maps.append(
            {"xt": xts[b], "wq": wqc, "wk": wkc, "wv": wvc, "wp": wpc, "cost": cost, "sint": sint}
        )
    return in_maps


def kernel(x, w_attn, w_proj):
    global _NC
    from concourse.bass_utils import run_bass_kernel_spmd

    if _NC is None:
        _NC = _build_nc()
    in_maps = make_in_maps(x, w_attn, w_proj)
    res = run_bass_kernel_spmd(_NC, in_maps, list(range(N_CORES))).results
    out = np.empty((B, T, C), dtype=np.float32)
    for b in range(B):
        out[b] = res[2 * b]["out"].astype(np.float32) + res[2 * b + 1]["out"].astype(
            np.float32
        )
    return out
